# revision 56
# baseline (speedup 1.0000x reference)
"""Trainium2 Bass kernel for causal multi-head self-attention with RoPE.

Problem: x[4,2048,1024], 16 heads, head_dim 64, causal, RoPE theta=1e4,
qkv proj + out proj.  Sharded over 8 cores: core c -> batch c//2, head
group c%2 (8 heads).  Host sums the two head-group partial outputs per
batch (the w_out all-reduce).

v3 design (vs v2): head-PAIR attention units so the two 64-contract
score matmuls land on PE row-tiles T0/T8 back-to-back and overlap
(2x score throughput), PV emitted BEFORE the next S so ready work is
never stuck behind an ACT-blocked S matmul (PE p-state stays warm),
shuffle matmuls decoupled from the qkv-group PSUM rotation, and a
per-pair (not per-head) softmax-denominator reciprocal chain.

Further scheduling work in this version: dependency-free PE warm-up
matmuls ride out the DVFS ramp (0.65->2.4GHz) while the first DMAs
are in flight; the startup weight/x stream is hand-interleaved on the
two HWDGE rings with batched multi-dim weight DMAs (the slow gpsimd
SWDGE only carries small tables); causal masks run as gpsimd
affine_selects to keep the DVE queue short (normalize muls gate PV
starts through the po-bank rotation); the out-projection of the last
query chunk is split so the pairs-0..2 accumulation overlaps the
final pair's reciprocal/broadcast chain.
"""
import numpy as np
import ml_dtypes

import concourse.bass as bass
import concourse.bacc as bacc
import concourse.mybir as mybir
import concourse.tile as tile

F32 = mybir.dt.float32
BF16 = mybir.dt.bfloat16
AF = mybir.ActivationFunctionType

THETA = 10000.0
S = 2048
D = 1024
NH = 8          # heads per core
DH = 64
EL = 512        # local head dims (NH*DH)
HALF = 1024     # tokens per QKV phase-1 half
PI_2 = 1.5707963267948966


def build_nc():
    nc = bacc.Bacc("TRN2", target_bir_lowering=False, debug=False)

    xT = nc.dram_tensor("xT", [D, S], BF16, kind="ExternalInput").ap()
    wqkT = nc.dram_tensor("wqkT", [D, 2 * EL], BF16, kind="ExternalInput").ap()
    wvT = nc.dram_tensor("wvT", [D, EL], BF16, kind="ExternalInput").ap()
    wout = nc.dram_tensor("wout", [EL, D], BF16, kind="ExternalInput").ap()
    posf = nc.dram_tensor("posf", [1, S], F32, kind="ExternalInput").ap()
    invf = nc.dram_tensor("invf", [128, 1], F32, kind="ExternalInput").ap()
    sgn = nc.dram_tensor("sgn", [128, 1], F32, kind="ExternalInput").ap()
    shufP = nc.dram_tensor("shufP", [128, 128], BF16, kind="ExternalInput").ap()
    y = nc.dram_tensor("y", [S, D], BF16, kind="ExternalOutput").ap()

    with tile.TileContext(nc) as tc:
        kernel_body(tc, xT, wqkT, wvT, wout, posf, invf, sgn, shufP, y)
    nc.compile()
    return nc


def kernel_body(tc, xT, wqkT, wvT, wout, posf, invf, sgn, shufP, y):
    with (
        tc.tile_pool(name="sb", bufs=1) as sb,
        tc.tile_pool(name="pp", bufs=1, space="PSUM") as pp,
    ):
        _kernel(tc, sb, pp, xT, wqkT, wvT, wout, posf, invf, sgn, shufP, y)


def _kernel(tc, sb, pp, xT, wqkT, wvT, wout, posf, invf, sgn, shufP, y):
    nc = tc.nc

    # ---- persistent tiles ----------------------------------------------
    qk_sb = [sb.tile([128, S], BF16, tag="qk", bufs=8, name=f"qk{t}")
             for t in range(8)]
    vtiles = [sb.tile([128, NH, 65], BF16, tag="v", bufs=16, name=f"v{t}")
              for t in range(16)]
    ctab = sb.tile([128, S], BF16, tag="cs", bufs=2, name="ctab")
    stab = sb.tile([128, S], BF16, tag="cs", bufs=2, name="stab")

    # ---- input DMAs (ordered for earliest PE start) --------------------
    # Batched multi-dim DMAs: the SWDGE queues cost ~600ns PER ISSUE, so
    # 8-tile weight groups and x halves move as single transfers.
    xTr = xT.rearrange("(a p) c -> p a c", p=128)      # [128, 8, 2048]
    wqkTr = wqkT.rearrange("(a p) c -> p a c", p=128)  # [128, 8, 1024]
    wvTr = wvT.rearrange("(a p) c -> p a c", p=128)    # [128, 8, 512]
    woutr = wout.rearrange("(a p) c -> p a c", p=128)  # [128, 4, 1024]

    # per-d xh tiles: separate tiles keep DMA->matmul dependencies exact
    # (slab DMAs into one big tile created false interval-overlap deps)
    xh = {}
    for H in (0, 1):
        xh[H] = [sb.tile([128, HALF], BF16, tag="xh", bufs=12, name=f"x{H}{d}")
                 for d in range(8)]

    def dma_xh(H):
        # only half 1 goes through here; both HWDGE rings, full-width
        s0 = H * HALF
        for d in range(8):
            eng = nc.sync if d % 2 == 0 else nc.scalar
            eng.dma_start(out=xh[H][d],
                          in_=xT[d * 128:(d + 1) * 128, s0:s0 + HALF])

    def dma_wqk(blk, ep):
        wcol = blk * EL + ep * 256
        w8 = sb.tile([128, 8, 256], BF16, tag="w", bufs=3, name="wqk")
        nc.scalar.dma_start(out=w8, in_=wqkTr[:, :, wcol:wcol + 256])
        return [w8[:, d, :] for d in range(8)]

    # --- PE warm-up: dependency-free matmuls on garbage SBUF so the PE's
    # DVFS ramp (0.65->2.4GHz over ~3us of continuous work) completes while
    # the first weight/x DMAs are still in flight
    warm_ps = pp.tile([128, 2, 512], F32, tag="ps", bufs=2, name="warm")
    for w in range(10):
        nc.tensor.matmul(warm_ps[:, w % 2, :],
                         lhsT=qk_sb[6][:, 0:128], rhs=qk_sb[7][:, 0:512],
                         start=True, stop=True)
    # dummy Exp as the FIRST activation: pins the exp_and_others table
    # (which also covers Copy) into slot 0 at startup -- otherwise the
    # table pass loads a copy-only set first and reloads mid-attention
    # (1.3us ACT stall right at the first exp)
    warm_act = sb.tile([128, 1], F32, tag="cvec", bufs=2, name="warmact")
    nc.scalar.activation(warm_act, qk_sb[6][:, 0:1], AF.Exp)

    # --- startup DMA schedule: both HWDGE rings (scalar + sync) hand-
    # interleaved so each weight/x chunk lands just ahead of the PE's
    # d-sweep; the slow gpsimd SWDGE only carries the small tables.
    w00 = sb.tile([128, 8, 256], BF16, tag="w", bufs=3, name="w00")
    w01 = sb.tile([128, 8, 256], BF16, tag="w", bufs=3, name="w01")
    nc.scalar.dma_start(out=w00[:, 0, :], in_=wqkTr[:, 0, 0:256])
    for d in (0, 2, 4, 6):
        nc.sync.dma_start(out=xh[0][d][:, 0:512],
                          in_=xT[d * 128:(d + 1) * 128, 0:512])
    nc.scalar.dma_start(out=xh[0][1][:, 0:512], in_=xT[128:256, 0:512])
    nc.scalar.dma_start(out=w00[:, 1:4, :], in_=wqkTr[:, 1:4, 0:256])
    nc.scalar.dma_start(out=xh[0][3][:, 0:512], in_=xT[384:512, 0:512])
    nc.scalar.dma_start(out=w00[:, 4:8, :], in_=wqkTr[:, 4:8, 0:256])
    nc.scalar.dma_start(out=xh[0][5][:, 0:512], in_=xT[640:768, 0:512])
    nc.scalar.dma_start(out=xh[0][7][:, 0:512], in_=xT[896:1024, 0:512])
    for d in (0, 2, 4, 6):
        nc.sync.dma_start(out=xh[0][d][:, 512:HALF],
                          in_=xT[d * 128:(d + 1) * 128, 512:HALF])
    for d in (1, 3):
        nc.scalar.dma_start(out=xh[0][d][:, 512:HALF],
                            in_=xT[d * 128:(d + 1) * 128, 512:HALF])
    nc.scalar.dma_start(out=w01, in_=wqkTr[:, :, 256:512])
    for d in (5, 7):
        nc.scalar.dma_start(out=xh[0][d][:, 512:HALF],
                            in_=xT[d * 128:(d + 1) * 128, 512:HALF])
    wts00 = [w00[:, d, :] for d in range(8)]
    wts01 = [w01[:, d, :] for d in range(8)]

    # small tables on the gpsimd queue (idle; consumed ~12us in)
    invf_sb = sb.tile([128, 1], F32, tag="cvec", bufs=2)
    nc.gpsimd.dma_start(out=invf_sb, in_=invf)
    sgn_sb = sb.tile([128, 1], F32, tag="cvec", bufs=2)
    nc.gpsimd.dma_start(out=sgn_sb, in_=sgn)
    shufP_sb = sb.tile([128, 128], BF16, tag="shufP", bufs=1)
    nc.gpsimd.dma_start(out=shufP_sb, in_=shufP)
    posf_sb = sb.tile([1, S], F32, tag="rt", bufs=2)
    nc.gpsimd.dma_start(out=posf_sb, in_=posf)

    wv8 = sb.tile([128, 8, EL], BF16, tag="wv", bufs=1, name="wv")
    nc.sync.dma_start(out=wv8[:, 0:4, :], in_=wvTr[:, 0:4, :])
    nc.scalar.dma_start(out=wv8[:, 4:8, :], in_=wvTr[:, 4:8, :])
    wv_sb = [wv8[:, d, :] for d in range(8)]

    # ---- phase 0: RoPE tables + v ones ---------------------------------
    for t in range(16):
        # softmax-denominator ones column (col 64 of each head's v tile)
        nc.gpsimd.memset(vtiles[t][:, :, 64:65], 1.0)

    pos_b = sb.tile([128, S], F32, tag="rt", bufs=2)
    nc.gpsimd.partition_broadcast(pos_b, posf_sb)
    angles = sb.tile([128, S], F32, tag="rt", bufs=2)
    nc.vector.tensor_scalar_mul(angles, pos_b, invf_sb)
    # range-reduce angles into [-pi, pi]:  k = round(angle / 2pi) via the
    # magic-constant trick, then 3-term Cody-Waite  x - k*2pi.
    TWO_PI = 6.283185307179586
    MAGIC = 1.5 * 2.0 ** 23
    kq = sb.tile([128, S], F32, tag="rt", bufs=2)
    nc.vector.tensor_scalar_mul(kq, angles, 1.0 / TWO_PI)
    nc.vector.tensor_scalar(kq, kq, MAGIC, MAGIC,
                            mybir.AluOpType.add, mybir.AluOpType.subtract)
    CW1 = 6.28125
    CW2 = float(np.float32(TWO_PI - CW1))
    CW3 = float(TWO_PI - CW1 - np.float64(np.float32(TWO_PI - CW1)))
    nc.vector.cody_waite_cascade(angles, angles, kq, CW1, CW2, CW3)
    nc.vector.add_range_wrap(kq, angles, 0.0, np.pi, TWO_PI)
    nc.scalar.activation(stab, kq, AF.Sin)
    nc.vector.add_range_wrap(angles, angles, PI_2, np.pi, TWO_PI)
    nc.scalar.activation(ctab, angles, AF.Sin)
    nc.vector.tensor_scalar_mul(stab, stab, sgn_sb)

    # ---- phase 1 building blocks ---------------------------------------
    def qk_ei(H, blk, ep, ei, wts):
        """16 qkv matmuls for one 128-row output block + PSUM->SBUF copy.
        Returns a closure that does the RoPE-partner shuffle (own PSUM
        alloc, decoupled from this group's rotation slot)."""
        s0 = H * HALF
        ps = pp.tile([128, 2, 512], F32, tag="ps", bufs=2, name="psqk")
        # sc-major then d-major: each matmul only needs one 512-col half of
        # one xh tile, so the PE streams right behind the x DMA
        for sc in range(2):
            for d in range(8):
                nc.tensor.matmul(
                    ps[:, sc, :], lhsT=wts[d][:, ei * 128:(ei + 1) * 128],
                    rhs=xh[H][d][:, sc * 512:(sc + 1) * 512],
                    start=(d == 0), stop=(d == 7))
        t = blk * 4 + ep * 2 + ei
        sl = qk_sb[t][:, s0:s0 + HALF]
        # alternate the PSUM->SBUF copy between ACT and DVE to balance queues
        if ei == 0:
            nc.scalar.copy(sl, ps.rearrange("p a b -> p (a b)"))
        else:
            nc.vector.tensor_copy(sl, ps.rearrange("p a b -> p (a b)"))

        def shuf(sl=sl, s0=s0):
            # RoPE partner values via a PE permutation matmul reading the
            # already-copied sl; allocates its own ps slot so the qkv
            # group's slot is free as soon as the sl copy drains.
            ps2 = pp.tile([128, 2, 512], F32, tag="ps", bufs=2, name="psshuf")
            for sc in range(2):
                nc.tensor.matmul(ps2[:, sc, :], lhsT=shufP_sb,
                                 rhs=sl[:, sc * 512:(sc + 1) * 512],
                                 start=True, stop=True)
            if H == 0:
                # fin runs inline, so the ps slot is released immediately
                # anyway: read the shuffle result straight from PSUM and
                # skip the staging copy
                def fin(sl=sl, ps2=ps2, s0=s0):
                    t1 = sb.tile([128, HALF], BF16, tag="rt1", bufs=2,
                                 name="t1")
                    sh = sb.tile([128, HALF], BF16, tag="rsh", bufs=2,
                                 name="shufB")
                    nc.vector.tensor_mul(t1, sl, ctab[:, s0:s0 + HALF])
                    nc.vector.tensor_mul(sh, ps2.rearrange("p a b -> p (a b)"),
                                         stab[:, s0:s0 + HALF])
                    nc.vector.tensor_add(sl, t1, sh)
                return fin
            shufB = sb.tile([128, HALF], BF16, tag="rsh", bufs=2, name="shufB")
            if ei == 1:
                nc.scalar.copy(shufB, ps2.rearrange("p a b -> p (a b)"))
            else:
                nc.vector.tensor_copy(shufB, ps2.rearrange("p a b -> p (a b)"))

            def fin(sl=sl, shufB=shufB, s0=s0):
                t1 = sb.tile([128, HALF], BF16, tag="rt1", bufs=2, name="t1")
                nc.vector.tensor_mul(t1, sl, ctab[:, s0:s0 + HALF])
                nc.vector.tensor_mul(shufB, shufB, stab[:, s0:s0 + HALF])
                nc.vector.tensor_add(sl, t1, shufB)
            return fin
        return shuf

    def v_pair(H, p):
        # half-0 uses the attention-idle po tag so the ps rotation serves
        # only the qkv chains
        if H == 0:
            pss = [pp.tile([128, 512], F32, tag="po", bufs=4, name="psv")
                   for _ in range(2)]
        else:
            ps = pp.tile([128, 2, 512], F32, tag="ps", bufs=2, name="psv")
            pss = [ps[:, 0, :], ps[:, 1, :]]
        for d in range(8):
            for i in range(2):
                st = 2 * p + i
                nc.tensor.matmul(
                    pss[i], lhsT=xh[H][d][:, st * 128:(st + 1) * 128],
                    rhs=wv_sb[d], start=(d == 0), stop=(d == 7))
        for i in range(2):
            st = 2 * p + i
            vt = vtiles[H * 8 + st]
            if i == 0:
                nc.scalar.copy(vt[:, :, 0:64],
                               pss[i].rearrange("p (h e) -> p h e", h=NH))
            else:
                nc.vector.tensor_copy(vt[:, :, 0:64],
                                      pss[i].rearrange("p (h e) -> p h e", h=NH))

    def warm(k):
        # dependency-free PE filler: phase 1 is x-arrival-bound, and any
        # PE idle gap drops the clock to 1.2GHz for the next 3us
        wp = pp.tile([128, 2, 512], F32, tag="ps", bufs=2, name="warmf")
        for w in range(k):
            nc.tensor.matmul(wp[:, w % 2, :],
                             lhsT=qk_sb[6][:, 0:128], rhs=qk_sb[7][:, 0:512],
                             start=True, stop=True)

    # ---- phase 1, half 0 -----------------------------------------------
    pre = {(0, 0): wts00, (0, 1): wts01}
    for blk in range(2):
        for ep in range(2):
            wts = pre.get((blk, ep)) or dma_wqk(blk, ep)
            shufs = [qk_ei(0, blk, ep, ei, wts) for ei in range(2)]
            if blk == 0:
                warm(4)
            v_pair(0, blk * 2 + ep)
            for sh in shufs:
                sh()()   # shuffle matmuls + copy, then RoPE fin inline

    dma_xh(1)
    wo8 = sb.tile([128, 4, D], BF16, tag="wout", bufs=1, name="wo")
    nc.scalar.dma_start(out=wo8, in_=woutr)
    wout_sb = [wo8[:, pair, :] for pair in range(NH // 2)]

    # ---- phase 2: head-pair attention units, global software pipeline --
    # Unit = one 128-key tile for one head pair: the two 64-contract score
    # matmuls (even head -> PE row tile T0, odd head -> T8) are adjacent
    # and independent so the array halves overlap.  Diag tiles first (max
    # mask slack), then mask-free full tiles.
    units = []
    for qc in range(4):
        nd = 4 * qc
        kis = list(range(nd, nd + 4)) + list(range(0, nd))

        def mk(p, idx):
            ki = kis[idx]
            return dict(qc=qc, p=p, ki=ki, rel=max(0, ki * 128 - qc * 512),
                        first=(idx == 0), last=(idx == len(kis) - 1),
                        diag=(ki >= nd))
        for p in range(4):
            for idx in range(len(kis)):
                units.append(mk(p, idx))
    n = len(units)

    # fillers before S-emission of unit index i
    fillers = {}
    rope_fins = []

    def run_qk1(blk, ep, ei, wts):
        shuf = qk_ei(1, blk, ep, ei, wts)

        def run_shuf(shuf=shuf):
            rope_fins.append(shuf())
        return run_shuf

    seq = []
    wts1 = {}
    for blk in range(2):
        for ep in range(2):
            def load(blk=blk, ep=ep):
                wts1[(blk, ep)] = dma_wqk(blk, ep)
            shuf_runs = []

            def ei0(blk=blk, ep=ep, shuf_runs=shuf_runs):
                shuf_runs.append(run_qk1(blk, ep, 0, wts1[(blk, ep)]))

            def ei1(blk=blk, ep=ep, shuf_runs=shuf_runs):
                shuf_runs.append(run_qk1(blk, ep, 1, wts1[(blk, ep)]))

            def shufs(shuf_runs=shuf_runs):
                for sr in shuf_runs:
                    sr()
            seq.append([load, ei0])
            seq.append([ei1])
            seq.append([lambda p=blk * 2 + ep: v_pair(1, p)])
            seq.append([shufs])
    for i, fs in enumerate(seq):
        fillers.setdefault(2 * i, []).extend(fs)

    po_map = {}
    ocs = {}

    def emit_S(u):
        qc, p, ki, rel = u['qc'], u['p'], u['ki'], u['rel']
        q0 = qc * 512
        qt = qk_sb[p]
        kt = qk_sb[4 + p]
        ps = pp.tile([128, 2, 512], F32, tag="ps", bufs=2, name="ps")
        pr = sb.tile([128, 2, 512], BF16, tag="pr", bufs=8, name="pr")
        u['pr'] = pr
        for par in range(2):
            jb = par * 64
            nc.tensor.matmul(
                ps[:, par, rel:512],
                lhsT=kt[jb:jb + 64, ki * 128:(ki + 1) * 128],
                rhs=qt[jb:jb + 64, q0 + rel:q0 + 512], start=True, stop=True)
        nc.scalar.activation(pr[:, :, rel:512], ps[:, :, rel:512],
                             AF.Exp, scale=0.125)
        if u['diag']:
            # causal mask on gpsimd (mostly idle): keeps the DVE queue
            # short so normalize muls - which gate PV starts through the
            # po rotation - run promptly
            nc.gpsimd.affine_select(
                out=pr[:, :, rel:rel + 128], in_=pr[:, :, rel:rel + 128],
                compare_op=mybir.AluOpType.is_ge, fill=0.0,
                base=0, channel_multiplier=-1, pattern=[[0, 2], [1, 128]])

    def emit_PV(u):
        qc, p, ki, rel = u['qc'], u['p'], u['ki'], u['rel']
        key = (qc, p)
        if key not in po_map:
            po_map[key] = (pp.tile([65, 512], F32, tag="po", bufs=4, name="poe"),
                           pp.tile([65, 512], F32, tag="po", bufs=4, name="poo"))
        poe, poo = po_map[key]
        pr = u['pr']
        nc.tensor.matmul(poe[:, rel:512], lhsT=vtiles[ki][:, 2 * p, :],
                         rhs=pr[:, 0, rel:512], start=u['first'], stop=u['last'])
        nc.tensor.matmul(poo[:, rel:512], lhsT=vtiles[ki][:, 2 * p + 1, :],
                         rhs=pr[:, 1, rel:512], start=u['first'], stop=u['last'])

    def normalize(qc, p):
        poe, poo = po_map.pop((qc, p))
        # custom-DVE ops read zeros from PSUM and misbehave off partition 0:
        # stage dens PSUM->SBUF (aligned), DMA to p0, recip there; the
        # broadcast runs in bf16 (halves the gpsimd bytes)
        den = sb.tile([65, 2, 512], F32, tag="rec", bufs=2, name="den")
        nc.vector.tensor_copy(den[64:65, 0, :], poe[64:65, :])
        nc.vector.tensor_copy(den[64:65, 1, :], poo[64:65, :])
        rec0a = sb.tile([1, 2, 512], F32, tag="rec0a", bufs=2, name="rec0a")
        nc.sync.dma_start(out=rec0a, in_=den[64:65, :, :])
        # per-parity recip/broadcast/mul so the stages pipeline across
        # DVE/gpsimd: the even head's po bank frees ~2us earlier
        rec0 = sb.tile([1, 2, 512], F32, tag="rec0", bufs=2, name="rec0")
        nc.vector.reciprocal_approx_fast(rec0[:, 0, :], rec0a[:, 0, :])
        nc.vector.reciprocal_approx_fast(rec0[:, 1, :], rec0a[:, 1, :])
        bca_e = sb.tile([64, 512], F32, tag="bca", bufs=4, name="bcae")
        bca_o = sb.tile([64, 512], F32, tag="bca", bufs=4, name="bcao")
        nc.gpsimd.partition_broadcast(bca_e, rec0[:, 0, :])
        nc.gpsimd.partition_broadcast(bca_o, rec0[:, 1, :])
        # heads are paired on 128 partitions for a full-contract projection;
        # odd head lands at partitions 64-127 via a local SBUF DMA (DVE
        # can't write off its operand partition base).
        ocp = sb.tile([128, 512], BF16, tag="oc", bufs=8, name=f"oc{qc}_{p}")
        ocs[(qc, p)] = ocp
        nc.vector.tensor_mul(ocp[0:64, :], poe[0:64, :], bca_e)
        oct = sb.tile([64, 512], BF16, tag="oct", bufs=2, name="oct")
        nc.vector.tensor_mul(oct, poo[0:64, :], bca_o)
        # last chunk's moves gate the tail projection: keep them off the
        # busy sync queue
        eng = nc.scalar if qc == 3 else nc.sync
        eng.dma_start(out=ocp[64:128, :], in_=oct)

    def proj_block(qcp, st, tail=False):
        ysb = sb.tile([128, D], BF16, tag="ysb", bufs=2, name="ysb")
        np_ = NH // 2
        for dmc in range(2):
            py = pp.tile([128, 512], F32, tag="po", bufs=4, name="py")
            for pr_ in range(np_):
                nc.tensor.matmul(
                    py, lhsT=ocs[(qcp, pr_)][:, st * 128:(st + 1) * 128],
                    rhs=wout_sb[pr_][:, dmc * 512:(dmc + 1) * 512],
                    start=(pr_ == 0), stop=(pr_ == np_ - 1))
            # tail: both ACT and DVE are free; split so the last copies
            # drain in parallel instead of serializing on one engine
            if tail and dmc == 0:
                nc.scalar.copy(ysb[:, dmc * 512:(dmc + 1) * 512], py)
            else:
                nc.vector.tensor_copy(ysb[:, dmc * 512:(dmc + 1) * 512], py)
        q0 = qcp * 512
        nc.sync.dma_start(out=y[q0 + st * 128:q0 + (st + 1) * 128, :], in_=ysb)

    # tail out-projection, split so the pairs-0..2 accumulation overlaps the
    # last pair's normalize chain (PE is otherwise idle for ~7us there)
    tail_py = {}

    def tail_early():
        # allocation order matters: st2/st3 take the free ps duals and run
        # immediately; st0/st1 rotate into po slots that only free as the
        # normalize chain progresses
        for st, tag in ((2, "ps"), (3, "ps"), (0, "po"), (1, "po")):
            if tag == "ps":
                py2 = pp.tile([128, 2, 512], F32, tag="ps", bufs=2, name="pyt")
                pys = [py2[:, 0, :], py2[:, 1, :]]
            else:
                pys = [pp.tile([128, 512], F32, tag="po", bufs=4, name="pyt")
                       for _ in range(2)]
            tail_py[st] = pys
            for dmc in range(2):
                for pr_ in range(3):
                    nc.tensor.matmul(
                        pys[dmc],
                        lhsT=ocs[(3, pr_)][:, st * 128:(st + 1) * 128],
                        rhs=wout_sb[pr_][:, dmc * 512:(dmc + 1) * 512],
                        start=(pr_ == 0), stop=False)

    def tail_finish():
        for st in (2, 3, 0, 1):
            pys = tail_py[st]
            ysb = sb.tile([128, D], BF16, tag="ysb", bufs=2, name="ysb")
            for dmc in range(2):
                nc.tensor.matmul(
                    pys[dmc], lhsT=ocs[(3, 3)][:, st * 128:(st + 1) * 128],
                    rhs=wout_sb[3][:, dmc * 512:(dmc + 1) * 512],
                    start=False, stop=True)
                if dmc == 0:
                    nc.scalar.copy(ysb[:, 0:512], pys[0])
                else:
                    nc.vector.tensor_copy(ysb[:, 512:D], pys[1])
            nc.sync.dma_start(out=y[1536 + st * 128:1536 + (st + 1) * 128, :],
                              in_=ysb)

    L = 6
    for i in range(n + L):
        # PV first: its pr is ready, so the PE never idles behind an S
        # matmul that's still waiting on the exp of an earlier unit.
        ip = i - L
        if ip >= 0:
            u = units[ip]
            emit_PV(u)
            if u['last']:
                qc, p = u['qc'], u['p']
                if (qc, p) == (3, 3):
                    # nothing but the tail partials here: their po slots
                    # must be the only contenders while the last normalize
                    # chain runs
                    tail_early()
                    normalize(qc, p)
                else:
                    normalize(qc, p)
                    if rope_fins:
                        # finish one deferred half-1 RoPE tile per pair-end
                        rope_fins.pop(0)()
                    # qc3: proj(2,*) shifted one pair early so the last
                    # pair's end is free for the tail overlap
                    if qc == 3:
                        if p == 0:
                            proj_block(2, 0)
                            proj_block(2, 1)
                        else:
                            proj_block(2, p + 1)
                    elif qc >= 1:
                        proj_block(qc - 1, p)
        if i < n:
            for f in fillers.get(i, []):
                f()
            emit_S(units[i])
    tail_finish()


# ======================= host-side sharding =============================

def _perm64():
    p = np.zeros(64, dtype=np.int64)
    for r in range(64):
        b, rem = divmod(r, 32)
        half, i = divmod(rem, 16)
        p[r] = 2 * (16 * b + i) + half
    return p


def _invf_sgn():
    f = np.zeros(128, dtype=np.int64)
    sg = np.zeros(128, dtype=np.float32)
    for p in range(128):
        r = p % 64
        f[p] = 16 * (r // 32) + (r % 16)
        sg[p] = -1.0 if (r % 32) < 16 else 1.0
    inv = (1.0 / THETA ** (2.0 * f / 64.0)).astype(np.float32)
    return inv.reshape(128, 1), sg.reshape(128, 1)


def make_in_maps(x, token_positions, w_qkv, w_out):
    BF = ml_dtypes.bfloat16
    x = np.asarray(x, dtype=np.float32)
    w_qkv = np.asarray(w_qkv, dtype=np.float32)
    w_out = np.asarray(w_out, dtype=np.float32)
    pos = np.asarray(token_positions)

    pm = _perm64()
    invf, sgn = _invf_sgn()
    posf = pos.astype(np.float32).reshape(1, S)
    shufP = np.zeros((128, 128), np.float32)
    for p in range(128):
        shufP[p, (p // 32) * 32 + (p % 32 + 16) % 32] = 1.0
    shufP = shufP.astype(BF)
    woutT = np.ascontiguousarray(w_out.T)

    xTs = [np.ascontiguousarray(x[b].T.astype(BF)) for b in range(4)]
    in_maps = []
    for c in range(8):
        b, g = c // 2, c % 2
        wq = w_qkv[g * EL:(g + 1) * EL]
        wk = w_qkv[D + g * EL:D + (g + 1) * EL]
        qrows = np.concatenate([wq[j * 64 + pm] for j in range(NH)], 0)
        krows = np.concatenate([wk[j * 64 + pm] for j in range(NH)], 0)
        wqkT = np.ascontiguousarray(np.concatenate([qrows, krows], 0).T.astype(BF))
        wvT = np.ascontiguousarray(
            w_qkv[2 * D + g * EL:2 * D + (g + 1) * EL].T.astype(BF))
        wout_c = np.ascontiguousarray(woutT[g * EL:(g + 1) * EL, :].astype(BF))
        in_maps.append(dict(xT=xTs[b], wqkT=wqkT, wvT=wvT, wout=wout_c,
                            posf=posf, invf=invf, sgn=sgn, shufP=shufP))
    return in_maps


def combine_outputs(results):
    """results: list of 8 dicts with 'y' [2048, 1024] bf16 -> [4, 2048, 1024]."""
    y = np.zeros((4, S, D), np.float32)
    for b in range(4):
        y[b] = (results[2 * b]["y"].astype(np.float32)
                + results[2 * b + 1]["y"].astype(np.float32))
    return y


def kernel(x, token_positions, w_qkv, w_out):
    from concourse.bass_utils import run_bass_kernel_spmd
    nc = build_nc()
    in_maps = make_in_maps(x, token_positions, w_qkv, w_out)
    res = run_bass_kernel_spmd(nc, in_maps, core_ids=list(range(8)))
    return combine_outputs(res.results)
